# revision 25
# baseline (speedup 1.0000x reference)
"""Trainium2 Bass kernel for the AR(t) recurrence problem.

Math: the recurrence  x_i = sum_j params[j] * x_{i-1-j} + bias  (i in [t, 2t))
is affine in the seed window:  out = inputs @ M + c,  with M, c functions of
params/bias only. M factors exactly as M = T2 @ (I + T1h) where T2 is the
lower-triangular Toeplitz of p_rev (= params reversed) and T1h the upper-
triangular Toeplitz of the AR impulse-response tail h~. The alternating-sign
params make T1h's error-carrying content coherent/smooth, so T1h is
numerically low-rank: a rank-128 SVD (host, randomized) captures it to
~4e-4 output error. Device compute per core (512 batch rows, fp8e4,
DoubleRow = 2 contraction k-tiles per PE pass):

  stage-1 (72 mm): psum1[ci] = f.T tiles       f.T = T2^T s.T  (triangular)
  stage-A ( 8 mm): psumY = (sA*T2@U)^T s.T  -> Y8               (rank-128)
  stage-B (16 mm): psum2[it] = (sB*V)^T Y8 + 16*f8[it]          (DoubleRow
                   pairs the V-tile with an exact 16*I fp8 tile, adding the
                   identity term in the same pass)
  psum2 = 2^14 (f + corr).T -> bf16 (Act/DVE alternating) -> DMA (sync ring)
  host: /2^14 + c

fp8 is ample precision: the data term has magnitude ~0.0155 rms while
|out| ~ 1.8 (dominated by c, exact). Measured max elementwise rel error
~3.6e-3 (tolerance 2e-2).
"""

import numpy as np
import ml_dtypes

B = 4096          # batch rows
T = 2048          # time steps == contraction length
NCORES = 8
BS = B // NCORES  # 512 rows per core
P = 128           # partitions
NJ = T // P       # 16 contraction tiles
NPAIR = NJ // 2   # 8 DoubleRow contraction pairs
RANK = 128        # low-rank correction rank
W_SCALE = 1024.0  # 2^10 on T2 tiles
SA = 128.0        # 2^7 on Pt (stage-A)
SB = 128.0        # 2^7 on V (stage-B); psum2 scale = W_SCALE * 16 = SA*SB = 2^14
OUT_SCALE = 16384.0

E4 = ml_dtypes.float8_e4m3

_cache = {}


def _build_and_compile():
    import concourse.mybir as mybir
    from concourse import bacc
    from concourse.tile import TileContext

    nc = bacc.Bacc(
        "TRN2",
        target_bir_lowering=False,
        debug=False,
        enable_asserts=False,
        num_devices=NCORES,
    )
    in8 = nc.dram_tensor("art_in8", [P, NJ, BS], mybir.dt.float8e4, kind="ExternalInput")
    w8 = nc.dram_tensor("art_w8", [P, NJ + 1, P], mybir.dt.float8e4, kind="ExternalInput")
    p8 = nc.dram_tensor("art_p8", [P, NJ, RANK], mybir.dt.float8e4, kind="ExternalInput")
    vi8 = nc.dram_tensor("art_vi8", [P, NJ + 1, P], mybir.dt.float8e4, kind="ExternalInput")
    # out.T in [partition, k-tile-pair-slot, b] layout so 2-tile SBUF groups DMA
    # with matching AP shapes; host reassembles.
    outT = nc.dram_tensor("art_outT", [P, NJ, BS], mybir.dt.bfloat16, kind="ExternalOutput")

    DR = mybir.MatmulPerfMode.DoubleRow

    with TileContext(nc) as tc:
        with (
            tc.tile_pool(name="wstk", bufs=1) as wpool,
            tc.tile_pool(name="pstk", bufs=1) as ppool,
            tc.tile_pool(name="vstk", bufs=1) as vpool,
            tc.tile_pool(name="warm", bufs=1) as wupool,
            tc.tile_pool(name="acts", bufs=NPAIR) as ipool,
            tc.tile_pool(name="yf", bufs=1) as yfpool,
            tc.tile_pool(name="outs", bufs=8) as opool,
            tc.tile_pool(name="ps1", bufs=3, space="PSUM") as f1pool,
            tc.tile_pool(name="ps2", bufs=5, space="PSUM") as c2pool,
        ):
            # W stack split into two tiles (overlapping at slot 2) so the
            # first stage-1 matmuls gate on a 48KB DMA, not the full stack.
            wta = wpool.tile([P, 3, P], mybir.dt.float8e4, name="wa_sb")
            wtb = wpool.tile([P, NJ - 1, P], mybir.dt.float8e4, name="wb_sb")
            pt_ = ppool.tile([P, NJ, RANK], mybir.dt.float8e4, name="p_sb")
            vt_ = vpool.tile([P, NJ + 1, P], mybir.dt.float8e4, name="v_sb")
            # three Y/f stacks (Y8 duplicated) so stage-B waves gate only on
            # their own quarter's f-casts and can run mid-stage-1.
            yfA = yfpool.tile([P, 9, BS], mybir.dt.float8e4, name="yfA_sb")
            yfC = yfpool.tile([P, 5, BS], mybir.dt.float8e4, name="yfC_sb")
            yfD = yfpool.tile([P, 5, BS], mybir.dt.float8e4, name="yfD_sb")
            wu = wupool.tile([P, 2, BS], mybir.dt.float8e4, name="wu_sb")
            nc.vector.memset(wu[:], 0.0)

            # input chunks batched 2 pairs per DMA (fewer descriptor-gen
            # serializations per ring); in_tiles[r] = (tile, pair-offset)
            in_tiles = [None] * NPAIR
            def in_dma(r, q):
                it = ipool.tile([P, 2, BS], mybir.dt.float8e4, tag="in", name=f"in_sb{r}")
                q.dma_start(out=it[:], in_=in8[:, 2 * r : 2 * r + 2, :])
                in_tiles[r] = it
            def in_rhs(r):
                return in_tiles[r][:]
            # first stage-1 tiles (ci=15,14) need only slots 0-2 of the W
            # stack; smallest DMA first so its completion sem fires earliest
            nc.sync.dma_start(out=wta[:], in_=w8[:, 0:3, :])
            in_dma(7, nc.sync)
            in_dma(6, nc.scalar)
            nc.sync.dma_start(out=pt_[:], in_=p8[:])
            in_dma(4, nc.scalar)
            nc.sync.dma_start(out=wtb[:], in_=w8[:, 2:17, :])
            in_dma(2, nc.scalar)
            in_dma(5, nc.sync)
            in_dma(0, nc.scalar)
            in_dma(3, nc.sync)
            nc.scalar.dma_start(out=vt_[:], in_=vi8[:])
            in_dma(1, nc.sync)

            # warm-up matmuls ramp the PE clock while the first chunks land
            wps = c2pool.tile([P, BS], mybir.dt.float32, tag="c", name="warmps")
            for _ in range(5):
                nc.tensor.matmul(wps[:], wu[:, :, :P], wu[:], perf_mode=DR)

            psY = c2pool.tile([P, BS], mybir.dt.float32, tag="c", name="psY")

            def stage_a(r):
                nc.tensor.matmul(
                    psY[:],
                    pt_[:, 2 * r : 2 * r + 2, :],
                    in_rhs(r),
                    start=(r == NPAIR - 1),
                    stop=(r == 0),
                    perf_mode=DR,
                )
                if r == 0:
                    nc.vector.tensor_scalar_mul(yfA[:, 0, :], psY[:], 1.0)
                    nc.scalar.copy(yfC[:, 0, :], psY[:])
                    nc.vector.tensor_scalar_mul(yfD[:, 0, :], psY[:], 1.0)

            def stage_1(ci, r):
                ps1 = f1pool.tile([P, BS], mybir.dt.float32, tag="f", name=f"f{ci}")
                for rp in range(r, NPAIR):
                    q = 2 * rp - ci + 1
                    lhsT = wta[:, q : q + 2, :] if q <= 1 else wtb[:, q - 2 : q, :]
                    nc.tensor.matmul(
                        ps1[:],
                        lhsT,
                        in_rhs(rp),
                        start=(rp == r),
                        stop=(rp == NPAIR - 1),
                        perf_mode=DR,
                    )
                # evacuate f.T tile to fp8, alternating engines
                if ci >= 8:
                    dst = yfA[:, ci - 7, :]
                elif ci >= 4:
                    dst = yfC[:, ci - 3, :]
                else:
                    dst = yfD[:, 1 + ci, :]
                if ci % 2 == 0:
                    nc.vector.tensor_scalar_mul(dst, ps1[:], 1.0)
                else:
                    nc.scalar.copy(dst, ps1[:])

            def stage_b_pair(hi):
                lo = hi - 1
                ot = opool.tile([P, 2, BS], mybir.dt.bfloat16, tag="o", name=f"o{lo}")
                for it, slot in ((hi, 1), (lo, 0)):
                    ps2 = c2pool.tile([P, BS], mybir.dt.float32, tag="c", name=f"c{it}")
                    if it >= 8:
                        rhs = yfA[:, 0 : it - 6 : it - 7, :]
                    elif it >= 4:
                        rhs = yfC[:, 0 : it - 2 : it - 3, :]
                    else:
                        rhs = yfD[:, 0 : it + 2 : it + 1, :]
                    nc.tensor.matmul(
                        ps2[:],
                        vt_[:, it : 17 : 16 - it, :],
                        rhs,
                        start=True,
                        stop=True,
                        perf_mode=DR,
                    )
                    if it % 2 == 1:
                        nc.scalar.copy(ot[:, slot, :], ps2[:])
                    else:
                        nc.vector.tensor_scalar_mul(ot[:, slot, :], ps2[:], 1.0)
                nc.sync.dma_start(out=outT[:, lo : lo + 2, :], in_=ot[:])

            # interleave: first stage-1 tiles gate on the earliest DMAs, then
            # all stage-A passes (Y8 ready early), stage-1 descending, with
            # stage-B tiles 8-15 inserted as soon as yfA is complete.
            stage_1(15, 7)
            stage_1(14, 7)
            stage_a(7)
            stage_1(13, 6)
            stage_1(12, 6)
            stage_a(6)
            stage_1(11, 5)
            stage_1(10, 5)
            for r in (5, 4, 3, 2, 1, 0):
                stage_a(r)
            stage_1(9, 4)
            stage_1(8, 4)
            stage_1(7, 3)
            stage_1(6, 3)
            for hi in (15, 13, 11, 9):
                stage_b_pair(hi)
            stage_1(5, 2)
            stage_1(4, 2)
            stage_1(3, 1)
            stage_1(2, 1)
            stage_b_pair(7)
            stage_b_pair(5)
            stage_1(1, 0)
            stage_1(0, 0)
            stage_b_pair(3)
            stage_b_pair(1)

    nc.compile()
    return nc


def _host_factors(params, bias):
    """All device operand tensors + c, from params/bias (float64 host math)."""
    t = T
    p_rev = params[::-1].astype(np.float64)

    # c: bias propagation through the recurrence
    b = np.float64(bias[0])
    u = np.zeros(t, np.float64)
    c = np.empty(t, np.float64)
    for k in range(t):
        nv = u @ p_rev + b
        c[k] = nv
        u = np.roll(u, -1)
        u[-1] = nv

    # h~: AR impulse response tail (h_0 = 1 excluded)
    a_full = np.concatenate([[0.0], params.astype(np.float64)])
    h = np.zeros(t)
    h[0] = 1.0
    for dd in range(1, t):
        h[dd] = a_full[1:dd + 1] @ h[dd - 1::-1][:dd]
    ht = h.copy()
    ht[0] = 0.0

    idx = np.arange(t)
    D = idx[:, None] - idx[None, :]
    T2 = np.where(D >= 0, p_rev[np.clip(D, 0, t - 1)], 0.0)
    T1h = np.where(-D >= 1, ht[np.clip(-D, 0, t - 1)], 0.0)

    # randomized SVD of the correction operator
    rng = np.random.default_rng(0)
    Q, _ = np.linalg.qr(T1h @ rng.standard_normal((t, RANK + 32)))
    u2, sig, vt = np.linalg.svd(Q.T @ T1h, full_matrices=False)
    U = (Q @ u2[:, :RANK]) * sig[:RANK]
    Vt = vt[:RANK]
    Pt = T2 @ U  # [t, RANK]

    # W stack: slot s=0..16 <-> tile-diagonal d=s-1; W[s][jw,kw]=1024*p_rev[128(s-1)+jw-kw]
    sidx = (128 * (np.arange(NJ + 1) - 1))[:, None, None] + idx[:P, None] - idx[None, :P]
    wvals = np.where(
        (sidx >= 0) & (sidx < t), (W_SCALE * p_rev)[np.clip(sidx, 0, t - 1)], 0.0
    )  # [17, 128, 128]
    w8 = np.ascontiguousarray(wvals.transpose(1, 0, 2).astype(np.float32)).astype(E4)

    p8 = np.ascontiguousarray(
        (SA * Pt).reshape(NJ, P, RANK).transpose(1, 0, 2).astype(np.float32)
    ).astype(E4)

    vi = np.empty((P, NJ + 1, P), np.float32)
    vi[:, :NJ, :] = (SB * Vt).reshape(P, NJ, P)
    vi[:, NJ, :] = 16.0 * np.eye(P, dtype=np.float32)
    vi8 = np.ascontiguousarray(vi).astype(E4)

    return w8, p8, vi8, c


def _make_in_maps(inputs, params, bias):
    w8, p8, vi8, c = _host_factors(params, bias)
    in8_full = inputs.astype(E4)
    in_maps = []
    for s in range(NCORES):
        shard = in8_full[s * BS : (s + 1) * BS, :]  # [BS, T]
        in8 = np.ascontiguousarray(shard.T.reshape(NJ, P, BS).transpose(1, 0, 2))
        in_maps.append({"art_in8": in8, "art_w8": w8, "art_p8": p8, "art_vi8": vi8})
    return in_maps, c


def run(inputs, params, bias, **spmd_kwargs):
    """Build in_maps, run the SPMD kernel, return (output, BassKernelResults)."""
    from concourse.bass_utils import run_bass_kernel_spmd

    if "nc" not in _cache:
        _cache["nc"] = _build_and_compile()
    nc = _cache["nc"]

    inputs = np.ascontiguousarray(np.asarray(inputs, dtype=np.float32))
    params = np.asarray(params, dtype=np.float32)
    bias = np.asarray(bias, dtype=np.float32)
    assert inputs.shape == (B, T), inputs.shape
    assert params.shape == (T,), params.shape
    in_maps, c = _make_in_maps(inputs, params, bias)
    res = run_bass_kernel_spmd(nc, in_maps, core_ids=list(range(NCORES)), **spmd_kwargs)
    scale = np.float32(1.0 / OUT_SCALE)
    c32 = c.astype(np.float32)
    outs = []
    for r in res.results:
        # art_outT [128, 16, 512]: [p, slot, b] with out.T row = 128*slot + p
        oT = r["art_outT"].astype(np.float32).transpose(1, 0, 2).reshape(T, BS)
        outs.append(oT.T * scale + c32[None, :])
    return np.concatenate(outs, axis=0), res


def kernel(inputs, params, bias):
    out, _ = run(inputs, params, bias)
    return out
